# revision 2
# baseline (speedup 1.0000x reference)
"""DoRA linear kernel for 8 Trainium2 NeuronCores.

out = (base_output + 2.0 * x @ lora_A^T @ lora_B^T) * magnitude / (||base_weight + 2.0 * lora_B @ lora_A||_row + eps)

Sharding:
  - tokens (B*S = 8192) data-parallel: 1024 per core (x, base_output, out)
  - base_weight / lora_B / magnitude row-parallel: 512 out_features per core
    (per-row norm fully local; mag_scale allgathered, 16KB collective)
  - lora_A and lora_B replicated for the activation path
"""

import sys

sys.path.insert(0, "/opt/trn_rl_repo")

import numpy as np

import concourse.bass as bass  # noqa: F401
import concourse.mybir as mybir
import concourse.tile as tile
from concourse import bacc
from concourse.bass_utils import run_bass_kernel_spmd
from concourse.masks import make_identity

N_CORES = 8
T, D, O, R = 8192, 4096, 4096, 64
T_LOC = T // N_CORES  # 1024 tokens per core
O_SH = O // N_CORES  # 512 weight rows per core
SCALING = 2.0
EPS = 1e-8
F32 = mybir.dt.float32

N_TB = T_LOC // 128  # 8 token blocks per core
N_OC = O_SH // 128  # 4 o-chunks per core (stage 0)
N_DC512 = D // 512  # 8 d-chunks of 512
N_DC128 = D // 128  # 32 d-chunks of 128
N_OCH = O // 512  # 8 output column chunks of 512

_CACHE: dict = {}


def _emit(nc, tc, aps):
    x_d = aps["x_shard"]
    base_d = aps["base_shard"]
    w_d = aps["w_shard"]
    b_sh_d = aps["b_shard"]
    b_full_d = aps["b_full"]
    a_d = aps["a_full"]
    mag_d = aps["mag_shard"]
    out_d = aps["out_shard"]

    import contextlib

    ctx = contextlib.ExitStack()
    with ctx:
        const = ctx.enter_context(tc.tile_pool(name="const", bufs=1))
        wpool = ctx.enter_context(tc.tile_pool(name="wpool", bufs=4))
        xpool = ctx.enter_context(tc.tile_pool(name="xpool", bufs=2))
        bpool = ctx.enter_context(tc.tile_pool(name="bpool", bufs=2))
        xtpool = ctx.enter_context(tc.tile_pool(name="xtpool", bufs=2))
        xapool = ctx.enter_context(tc.tile_pool(name="xapool", bufs=2))
        opool = ctx.enter_context(tc.tile_pool(name="opool", bufs=2))
        scpool = ctx.enter_context(tc.tile_pool(name="scpool", bufs=2))
        p_u = ctx.enter_context(tc.tile_pool(name="p_u", bufs=1, space="PSUM"))
        p_t = ctx.enter_context(tc.tile_pool(name="p_t", bufs=2, space="PSUM"))
        p_xa = ctx.enter_context(tc.tile_pool(name="p_xa", bufs=1, space="PSUM"))
        p_o = ctx.enter_context(tc.tile_pool(name="p_o", bufs=4, space="PSUM"))
        dram = ctx.enter_context(tc.tile_pool(name="dram", bufs=1, space="DRAM"))

        # ---- constants / identity
        ident = const.tile([128, 128], F32)
        make_identity(nc, ident[:])
        ones1 = const.tile([1, 128], F32)
        nc.gpsimd.memset(ones1[:], 1.0)

        # ---- small input DMAs (sync ring, ahead of everything)
        a_sb = const.tile([R, D], F32)  # lora_A natural [64, 4096]
        nc.sync.dma_start(a_sb[:], a_d[:])
        bf_sb = const.tile([128, 32, R], F32)  # lora_B rows chunked
        nc.sync.dma_start(bf_sb[:], b_full_d.rearrange("(c p) r -> p c r", p=128))
        bs_sb = const.tile([128, 4, R], F32)  # lora_B shard rows chunked
        nc.sync.dma_start(bs_sb[:], b_sh_d.rearrange("(c p) r -> p c r", p=128))
        magsh_sb = const.tile([128, 4], F32)  # magnitude shard [p, oc]
        nc.sync.dma_start(magsh_sb[:], mag_d.rearrange("(oc p) -> p oc", p=128))

        # ---- x token-block 0 early, then W, then base 0
        x_r = x_d.rearrange("(tb p) d -> tb p d", p=128)
        base_r = base_d.rearrange("(tb p) d -> tb p d", p=128)
        out_r = out_d.rearrange("(tb p) d -> tb p d", p=128)
        w_r = w_d.rearrange("(oc p) d -> oc p d", p=128)

        x_tiles = {}
        base_tiles = {}

        def load_x(tb, h):
            t = xpool.tile([128, 2048], F32, tag="x")
            nc.sync.dma_start(t[:], x_r[tb, :, 2048 * h : 2048 * (h + 1)])
            x_tiles[(tb, h)] = t

        def load_base(tb, h):
            t = bpool.tile([128, 2048], F32, tag="base")
            nc.sync.dma_start(t[:], base_r[tb, :, 2048 * h : 2048 * (h + 1)])
            base_tiles[(tb, h)] = t

        load_x(0, 0)
        load_x(0, 1)
        w_tiles = []
        for oc in range(N_OC):
            wt = wpool.tile([128, D], F32, tag="w")
            nc.sync.dma_start(wt[:], w_r[oc])
            w_tiles.append(wt)
        load_base(0, 0)
        load_base(0, 1)

        # ---- preprocessing transposes
        # A^T chunks: At_sb[:, 64*dc : 64*dc+64] = A[:, 128*dc:...]^T  [128, 64]
        at_sb = const.tile([128, 64 * N_DC128], F32)
        for g in range(4):  # 8 chunks per psum tile
            pt = p_t.tile([128, 512], F32, tag="pt")
            for j in range(8):
                dc = 8 * g + j
                nc.tensor.transpose(
                    pt[:, 64 * j : 64 * (j + 1)],
                    a_sb[:, 128 * dc : 128 * (dc + 1)],
                    ident[0:R, 0:R],
                )
            nc.scalar.copy(at_sb[:, 512 * g : 512 * (g + 1)], pt[:])

        # B2f^T = 2 * lora_B^T  [64, 4096]
        b2ft_sb = const.tile([R, O], F32)
        for g in range(8):  # 4 chunks of [64, 128] per psum tile
            pt = p_t.tile([128, 512], F32, tag="pt")
            for j in range(4):
                c = 4 * g + j
                nc.tensor.transpose(
                    pt[0:R, 128 * j : 128 * (j + 1)], bf_sb[:, c, :], ident[:]
                )
            nc.scalar.mul(b2ft_sb[:, 512 * g : 512 * (g + 1)], pt[0:R, :], SCALING)

        # B2s^T = 2 * (lora_B shard)^T  [64, 512]
        b2st_sb = const.tile([R, O_SH], F32)
        pt = p_t.tile([128, 512], F32, tag="pt")
        for j in range(4):
            nc.tensor.transpose(
                pt[0:R, 128 * j : 128 * (j + 1)], bs_sb[:, j, :], ident[:]
            )
        nc.scalar.mul(b2st_sb[:], pt[0:R, :], SCALING)

        # ---- stage 0: mag_scale shard = mag / (||W + 2 B A||_row + eps)
        ss_sb = const.tile([128, N_OC, N_DC512], F32)  # per-(oc,dc) sum of squares
        magsc_sb = const.tile([128, 4], F32)
        for oc in range(N_OC):
            for dc in range(N_DC512):
                pu = p_u.tile([128, 512], F32, tag="pu")
                nc.tensor.matmul(
                    pu[:],
                    b2st_sb[:, 128 * oc : 128 * (oc + 1)],
                    a_sb[:, 512 * dc : 512 * (dc + 1)],
                    start=True,
                    stop=False,
                )
                nc.tensor.matmul(
                    pu[:],
                    ident[:],
                    w_tiles[oc][:, 512 * dc : 512 * (dc + 1)],
                    start=False,
                    stop=True,
                )
                sq = scpool.tile([128, 512], F32, tag="sq")
                nc.scalar.activation(
                    sq[:],
                    pu[:],
                    mybir.ActivationFunctionType.Square,
                    accum_out=ss_sb[:, oc, dc : dc + 1],
                )
            ssum = scpool.tile([128, 1], F32, tag="ssum")
            nc.vector.tensor_reduce(
                ssum[:], ss_sb[:, oc, :], axis=mybir.AxisListType.X, op=mybir.AluOpType.add
            )
            nrm = scpool.tile([128, 1], F32, tag="nrm")
            nc.scalar.sqrt(nrm[:], ssum[:])
            nc.vector.tensor_scalar_add(nrm[:], nrm[:], EPS)
            rinv = scpool.tile([128, 1], F32, tag="rinv")
            nc.vector.reciprocal(rinv[:], nrm[:])
            nc.vector.tensor_tensor(
                out=magsc_sb[:, oc : oc + 1],
                in0=rinv[:],
                in1=magsh_sb[:, oc : oc + 1],
                op=mybir.AluOpType.mult,
            )

        # ---- allgather mag_scale -> broadcast tile [128, 4096]
        cc_in = dram.tile([O_SH], F32)
        cc_out = dram.tile([O], F32, addr_space="Shared")
        nc.gpsimd.dma_start(cc_in.rearrange("(oc p) -> p oc", p=128), magsc_sb[:])
        nc.gpsimd.collective_compute(
            "AllGather",
            mybir.AluOpType.bypass,
            replica_groups=[list(range(N_CORES))],
            ins=[cc_in[:]],
            outs=[cc_out[:]],
        )
        magrow_sb = const.tile([1, O], F32)
        nc.gpsimd.dma_start(magrow_sb[:], cc_out[None, :])
        magb_sb = const.tile([128, O], F32)
        for j in range(N_OCH):
            pb = p_o.tile([128, 512], F32, tag="po")
            nc.tensor.matmul(
                pb[:],
                ones1[:],
                magrow_sb[:, 512 * j : 512 * (j + 1)],
                start=True,
                stop=True,
            )
            nc.scalar.copy(magb_sb[:, 512 * j : 512 * (j + 1)], pb[:])

        # ---- main loop over token blocks
        for tb in range(N_TB):
            if tb + 1 < N_TB:  # prefetch next block's DMAs onto the ring
                load_x(tb + 1, 0)
                load_x(tb + 1, 1)
                load_base(tb + 1, 0)
                load_base(tb + 1, 1)

            # stage 1: xa^T[64, 128] = A @ x_tb^T via PE-transposed x chunks
            pxa = p_xa.tile([R, 128], F32, tag="pxa")
            for h in range(2):
                xh = x_tiles.pop((tb, h))
                for g in range(4):  # 4 groups of 4 d-chunks of 128
                    pt = p_t.tile([128, 512], F32, tag="pt")
                    for j in range(4):
                        nc.tensor.transpose(
                            pt[:, 128 * j : 128 * (j + 1)],
                            xh[:, 128 * (4 * g + j) : 128 * (4 * g + j + 1)],
                            ident[:],
                        )
                    xt = xtpool.tile([128, 512], F32, tag="xt")
                    nc.scalar.copy(xt[:], pt[:])
                    for j in range(4):
                        dc = 16 * h + 4 * g + j
                        nc.tensor.matmul(
                            pxa[:],
                            at_sb[:, 64 * dc : 64 * (dc + 1)],
                            xt[:, 128 * j : 128 * (j + 1)],
                            start=(dc == 0),
                            stop=(dc == N_DC128 - 1),
                        )
            xa_sb = xapool.tile([R, 128], F32, tag="xa")
            nc.scalar.copy(xa_sb[:], pxa[:])

            # stage 2 + epilogue: out = (base + xa^T.T @ B2^T) * mag_bcast
            for h in range(2):
                bh = base_tiles.pop((tb, h))
                osb = opool.tile([128, 2048], F32, tag="o")
                pos = [
                    p_o.tile([128, 512], F32, tag="po", name=f"po_{tb}_{h}_{j}")
                    for j in range(4)
                ]
                for j in range(4):
                    och = 4 * h + j
                    nc.tensor.matmul(
                        pos[j][:],
                        xa_sb[:],
                        b2ft_sb[:, 512 * och : 512 * (och + 1)],
                        start=True,
                        stop=False,
                    )
                for j in range(4):
                    nc.tensor.matmul(
                        pos[j][:],
                        ident[:],
                        bh[:, 512 * j : 512 * (j + 1)],
                        start=False,
                        stop=True,
                    )
                for j in range(4):
                    och = 4 * h + j
                    nc.vector.tensor_tensor(
                        out=osb[:, 512 * j : 512 * (j + 1)],
                        in0=pos[j][:],
                        in1=magb_sb[:, 512 * och : 512 * (och + 1)],
                        op=mybir.AluOpType.mult,
                    )
                nc.scalar.dma_start(
                    out_r[tb, :, 2048 * h : 2048 * (h + 1)], osb[:]
                )


def _build():
    nc = bacc.Bacc(
        "TRN2", target_bir_lowering=False, debug=False, num_devices=N_CORES
    )
    aps = {
        "x_shard": nc.dram_tensor("x_shard", [T_LOC, D], F32, kind="ExternalInput").ap(),
        "base_shard": nc.dram_tensor(
            "base_shard", [T_LOC, O], F32, kind="ExternalInput"
        ).ap(),
        "w_shard": nc.dram_tensor("w_shard", [O_SH, D], F32, kind="ExternalInput").ap(),
        "b_shard": nc.dram_tensor("b_shard", [O_SH, R], F32, kind="ExternalInput").ap(),
        "b_full": nc.dram_tensor("b_full", [O, R], F32, kind="ExternalInput").ap(),
        "a_full": nc.dram_tensor("a_full", [R, D], F32, kind="ExternalInput").ap(),
        "mag_shard": nc.dram_tensor(
            "mag_shard", [O_SH], F32, kind="ExternalInput"
        ).ap(),
        "out_shard": nc.dram_tensor(
            "out_shard", [T_LOC, O], F32, kind="ExternalOutput"
        ).ap(),
    }
    with tile.TileContext(nc) as tc:
        _emit(nc, tc, aps)
    nc.compile()
    return nc


def run(inputs: dict, trace: bool = False):
    """Run the SPMD kernel on full inputs; returns (full_output, BassKernelResults)."""
    if "nc" not in _CACHE:
        _CACHE["nc"] = _build()
    nc = _CACHE["nc"]

    x = np.asarray(inputs["x"], dtype=np.float32).reshape(T, D)
    base = np.asarray(inputs["base_output"], dtype=np.float32).reshape(T, O)
    w = np.asarray(inputs["base_weight"], dtype=np.float32)
    a = np.ascontiguousarray(np.asarray(inputs["lora_A"], dtype=np.float32))
    b = np.ascontiguousarray(np.asarray(inputs["lora_B"], dtype=np.float32))
    mag = np.asarray(inputs["magnitude"], dtype=np.float32)

    in_maps = []
    for c in range(N_CORES):
        in_maps.append(
            {
                "x_shard": np.ascontiguousarray(x[c * T_LOC : (c + 1) * T_LOC]),
                "base_shard": np.ascontiguousarray(base[c * T_LOC : (c + 1) * T_LOC]),
                "w_shard": np.ascontiguousarray(w[c * O_SH : (c + 1) * O_SH]),
                "b_shard": np.ascontiguousarray(b[c * O_SH : (c + 1) * O_SH]),
                "b_full": b,
                "a_full": a,
                "mag_shard": np.ascontiguousarray(mag[c * O_SH : (c + 1) * O_SH]),
            }
        )

    res = run_bass_kernel_spmd(
        nc, in_maps, core_ids=list(range(N_CORES)), trace=trace
    )
    out = np.concatenate(
        [res.results[c]["out_shard"] for c in range(N_CORES)], axis=0
    )
    return out, res


def kernel(**inputs) -> np.ndarray:
    x = inputs["x"]
    out, _ = run(inputs)
    return out.reshape(x.shape[0], x.shape[1], O).astype(np.float32)


# revision 7
# speedup vs baseline: 1.4103x; 1.4103x over previous
"""DoRA linear kernel for 8 Trainium2 NeuronCores.

out = (base_output + 2.0 * x @ lora_A^T @ lora_B^T) * magnitude / (||base_weight + 2.0 * lora_B @ lora_A||_row + eps)

Sharding:
  - tokens (B*S = 8192) data-parallel: 1024 per core (x, base_output, out)
  - base_weight / lora_B / magnitude row-parallel: 512 out_features per core
    (per-row norm fully local; mag_scale allgathered, 16KB collective)
  - lora_A and lora_B replicated for the activation path

Precision: the low-rank delta path (x @ A^T @ B^T and B@A) runs in bf16 on
the PE (fp32 matmuls are dual-pass LOW_HIGH = 2x slower); the base_output
add, base_weight norm accumulation, and magnitude rescale stay fp32.
"""

import sys

sys.path.insert(0, "/opt/trn_rl_repo")

import numpy as np

import concourse.bass as bass  # noqa: F401
import concourse.mybir as mybir
import concourse.tile as tile
from concourse import bacc
from concourse.bass_utils import run_bass_kernel_spmd
from concourse.masks import make_identity

N_CORES = 8
T, D, O, R = 8192, 4096, 4096, 64
T_LOC = T // N_CORES  # 1024 tokens per core
O_SH = O // N_CORES  # 512 weight rows per core
SCALING = 2.0
EPS = 1e-8
F32 = mybir.dt.float32
BF16 = mybir.dt.bfloat16

N_TB = T_LOC // 128  # 8 token blocks per core
N_OC = O_SH // 128  # 4 o-chunks per core (stage 0)
N_DC512 = D // 512  # 8 d-chunks of 512
N_DC128 = D // 128  # 32 d-chunks of 128
N_OCH = O // 512  # 8 output column chunks of 512

_CACHE: dict = {}


def _emit(nc, tc, aps):
    x_d = aps["x_shard"]
    base_d = aps["base_shard"]
    w_d = aps["w_shard"]
    b_sh_d = aps["b_shard"]
    b_full_d = aps["b_full"]
    a_d = aps["a_full"]
    mag_d = aps["mag_shard"]
    out_d = aps["out_shard"]

    import contextlib

    ctx = contextlib.ExitStack()
    with ctx:
        const = ctx.enter_context(tc.tile_pool(name="const", bufs=1))
        wpool = ctx.enter_context(tc.tile_pool(name="wpool", bufs=2))
        xpool = ctx.enter_context(tc.tile_pool(name="xpool", bufs=2))
        bpool = ctx.enter_context(tc.tile_pool(name="bpool", bufs=2))
        xtpool = ctx.enter_context(tc.tile_pool(name="xtpool", bufs=2))
        xapool = ctx.enter_context(tc.tile_pool(name="xapool", bufs=2))
        opool = ctx.enter_context(tc.tile_pool(name="opool", bufs=2))
        scpool = ctx.enter_context(tc.tile_pool(name="scpool", bufs=2))
        p_u = ctx.enter_context(tc.tile_pool(name="p_u", bufs=1, space="PSUM"))
        p_t = ctx.enter_context(tc.tile_pool(name="p_t", bufs=2, space="PSUM"))
        p_xa = ctx.enter_context(tc.tile_pool(name="p_xa", bufs=1, space="PSUM"))
        p_o = ctx.enter_context(tc.tile_pool(name="p_o", bufs=4, space="PSUM"))
        dram = ctx.enter_context(tc.tile_pool(name="dram", bufs=1, space="DRAM"))

        # ---- constants / identity
        ident = const.tile([128, 128], BF16)
        make_identity(nc, ident[:])
        ones1 = const.tile([1, 128], F32)
        nc.gpsimd.memset(ones1[:], 1.0)

        # ---- small input DMAs (bf16 cast loads go via SWDGE)
        a_sb = const.tile([R, D], BF16)  # lora_A natural [64, 4096]
        nc.gpsimd.dma_start(a_sb[:], a_d[:])
        bf_sb = const.tile([128, 32, R], BF16)  # lora_B rows chunked
        nc.gpsimd.dma_start(bf_sb[:], b_full_d.rearrange("(c p) r -> p c r", p=128))
        bs_sb = const.tile([128, 4, R], BF16)  # lora_B shard rows chunked
        nc.gpsimd.dma_start(bs_sb[:], b_sh_d.rearrange("(c p) r -> p c r", p=128))
        magsh_sb = const.tile([128, 4], F32)  # magnitude shard [p, oc]
        nc.sync.dma_start(magsh_sb[:], mag_d.rearrange("(oc p) -> p oc", p=128))

        # ---- big DMAs: x (bf16 cast, SWDGE), W + base (fp32, sync HWDGE)
        x_r = x_d.rearrange("(tb p) d -> tb p d", p=128)
        base_r = base_d.rearrange("(tb p) d -> tb p d", p=128)
        out_r = out_d.rearrange("(tb p) d -> tb p d", p=128)
        w_r = w_d.rearrange("(oc p) d -> oc p d", p=128)

        x_tiles = {}
        base_tiles = {}

        def load_x(tb):
            t = xpool.tile([128, D], BF16, tag="x")
            nc.gpsimd.dma_start(t[:], x_r[tb])
            x_tiles[tb] = t

        def load_base(tb):
            t = bpool.tile([128, D], F32, tag="base")
            nc.sync.dma_start(t[:], base_r[tb])
            base_tiles[tb] = t

        load_x(0)
        load_x(1)
        w_tiles = []
        for oc in range(N_OC):
            wt = wpool.tile([128, D], F32, tag="w", name=f"w_{oc}")
            nc.sync.dma_start(wt[:], w_r[oc])
            w_tiles.append(wt)
        load_base(0)
        load_base(1)

        # ---- preprocessing transposes (all bf16)
        # A^T chunks: At_sb[:, 64*dc : 64*dc+64] = A[:, 128*dc:...]^T  [128, 64]
        at_sb = const.tile([128, 64 * N_DC128], BF16)
        for g in range(4):  # 8 chunks per psum tile
            pt = p_t.tile([128, 512], BF16, tag="pt", name=f"pta_{g}")
            for j in range(8):
                dc = 8 * g + j
                nc.tensor.transpose(
                    pt[:, 64 * j : 64 * (j + 1)],
                    a_sb[:, 128 * dc : 128 * (dc + 1)],
                    ident[0:R, 0:R],
                )
            nc.scalar.copy(at_sb[:, 512 * g : 512 * (g + 1)], pt[:])

        # B2f^T = 2 * lora_B^T  [64, 4096] bf16
        b2ft_sb = const.tile([R, O], BF16)
        for g in range(8):  # 4 chunks of [64, 128] per psum tile
            pt = p_t.tile([128, 512], BF16, tag="pt", name=f"ptb_{g}")
            for j in range(4):
                c = 4 * g + j
                nc.tensor.transpose(
                    pt[0:R, 128 * j : 128 * (j + 1)], bf_sb[:, c, :], ident[:]
                )
            nc.scalar.mul(b2ft_sb[:, 512 * g : 512 * (g + 1)], pt[0:R, :], SCALING)

        # B2s^T = 2 * (lora_B shard)^T  [64, 512] bf16
        b2st_sb = const.tile([R, O_SH], BF16)
        ptc = p_t.tile([128, 512], BF16, tag="pt")
        for j in range(4):
            nc.tensor.transpose(
                ptc[0:R, 128 * j : 128 * (j + 1)], bs_sb[:, j, :], ident[:]
            )
        nc.scalar.mul(b2st_sb[:], ptc[0:R, :], SCALING)

        # ---- stage 0: mag_scale shard = mag / (||W + 2 B A||_row + eps)
        ss_sb = const.tile([128, N_OC, N_DC512], F32)  # per-(oc,dc) sum of squares
        magsc_sb = const.tile([128, 4], F32)
        for oc in range(N_OC):
            for dc in range(N_DC512):
                pu = p_u.tile([128, 512], F32, tag="pu")
                nc.tensor.matmul(
                    pu[:],
                    b2st_sb[:, 128 * oc : 128 * (oc + 1)],
                    a_sb[:, 512 * dc : 512 * (dc + 1)],
                    start=True,
                    stop=True,
                )
                u_sc = scpool.tile([128, 512], F32, tag="u")
                nc.vector.tensor_tensor(
                    out=u_sc[:],
                    in0=pu[:],
                    in1=w_tiles[oc][:, 512 * dc : 512 * (dc + 1)],
                    op=mybir.AluOpType.add,
                )
                sq = scpool.tile([128, 512], BF16, tag="sq")
                nc.scalar.activation(
                    sq[:],
                    u_sc[:],
                    mybir.ActivationFunctionType.Square,
                    accum_out=ss_sb[:, oc, dc : dc + 1],
                )
            ssum = scpool.tile([128, 1], F32, tag="ssum")
            nc.vector.tensor_reduce(
                ssum[:], ss_sb[:, oc, :], axis=mybir.AxisListType.X, op=mybir.AluOpType.add
            )
            nrm = scpool.tile([128, 1], F32, tag="nrm")
            nc.scalar.sqrt(nrm[:], ssum[:])
            nc.vector.tensor_scalar_add(nrm[:], nrm[:], EPS)
            rinv = scpool.tile([128, 1], F32, tag="rinv")
            nc.vector.reciprocal(rinv[:], nrm[:])
            nc.vector.tensor_tensor(
                out=magsc_sb[:, oc : oc + 1],
                in0=rinv[:],
                in1=magsh_sb[:, oc : oc + 1],
                op=mybir.AluOpType.mult,
            )

        # ---- allgather mag_scale -> broadcast tile [128, 4096] fp32
        cc_in = dram.tile([O_SH], F32)
        cc_out = dram.tile([O], F32, addr_space="Shared")
        nc.gpsimd.dma_start(cc_in.rearrange("(oc p) -> p oc", p=128), magsc_sb[:])
        nc.gpsimd.collective_compute(
            "AllGather",
            mybir.AluOpType.bypass,
            replica_groups=[list(range(N_CORES))],
            ins=[cc_in[:]],
            outs=[cc_out[:]],
        )
        magrow_sb = const.tile([1, O], F32)
        nc.gpsimd.dma_start(magrow_sb[:], cc_out[None, :])
        magb_sb = const.tile([128, O], F32)
        for j in range(N_OCH):
            pb = p_o.tile([128, 512], F32, tag="po", name=f"pb_{j}")
            nc.tensor.matmul(
                pb[:],
                ones1[:],
                magrow_sb[:, 512 * j : 512 * (j + 1)],
                start=True,
                stop=True,
            )
            nc.scalar.copy(magb_sb[:, 512 * j : 512 * (j + 1)], pb[:])

        # ---- main loop over token blocks
        for tb in range(N_TB):
            if tb + 2 < N_TB:  # prefetch (2 tiles already in flight)
                load_x(tb + 2)
            if tb + 2 < N_TB:
                load_base(tb + 2)

            # stage 1: xa^T[64, 128] = A @ x_tb^T via PE-transposed x chunks
            pxa = p_xa.tile([R, 128], F32, tag="pxa")
            xh = x_tiles.pop(tb)
            for g in range(8):  # 8 groups of 4 d-chunks of 128
                pt = p_t.tile([128, 512], BF16, tag="pt")
                for j in range(4):
                    nc.tensor.transpose(
                        pt[:, 128 * j : 128 * (j + 1)],
                        xh[:, 128 * (4 * g + j) : 128 * (4 * g + j + 1)],
                        ident[:],
                    )
                xt = xtpool.tile([128, 512], BF16, tag="xt")
                nc.scalar.copy(xt[:], pt[:])
                for j in range(4):
                    dc = 4 * g + j
                    nc.tensor.matmul(
                        pxa[:],
                        at_sb[:, 64 * dc : 64 * (dc + 1)],
                        xt[:, 128 * j : 128 * (j + 1)],
                        start=(dc == 0),
                        stop=(dc == N_DC128 - 1),
                    )
            xa_sb = xapool.tile([R, 128], BF16, tag="xa")
            nc.scalar.copy(xa_sb[:], pxa[:])

            # stage 2 + epilogue: out = (base + xa^T.T @ B2^T) * mag_bcast
            bh = base_tiles.pop(tb)
            osb = opool.tile([128, D], F32, tag="o")
            for h in range(2):
                pos = [
                    p_o.tile([128, 512], F32, tag="po", name=f"po_{tb}_{h}_{j}")
                    for j in range(4)
                ]
                for j in range(4):
                    och = 4 * h + j
                    nc.tensor.matmul(
                        pos[j][:],
                        xa_sb[:],
                        b2ft_sb[:, 512 * och : 512 * (och + 1)],
                        start=True,
                        stop=True,
                    )
                for j in range(4):
                    och = 4 * h + j
                    comb = scpool.tile([128, 512], F32, tag="comb")
                    nc.vector.tensor_tensor(
                        out=comb[:],
                        in0=pos[j][:],
                        in1=bh[:, 512 * och : 512 * (och + 1)],
                        op=mybir.AluOpType.add,
                    )
                    nc.vector.tensor_tensor(
                        out=osb[:, 512 * och : 512 * (och + 1)],
                        in0=comb[:],
                        in1=magb_sb[:, 512 * och : 512 * (och + 1)],
                        op=mybir.AluOpType.mult,
                    )
            nc.scalar.dma_start(out_r[tb], osb[:])


def _build():
    nc = bacc.Bacc(
        "TRN2", target_bir_lowering=False, debug=False, num_devices=N_CORES
    )
    aps = {
        "x_shard": nc.dram_tensor("x_shard", [T_LOC, D], F32, kind="ExternalInput").ap(),
        "base_shard": nc.dram_tensor(
            "base_shard", [T_LOC, O], F32, kind="ExternalInput"
        ).ap(),
        "w_shard": nc.dram_tensor("w_shard", [O_SH, D], F32, kind="ExternalInput").ap(),
        "b_shard": nc.dram_tensor("b_shard", [O_SH, R], F32, kind="ExternalInput").ap(),
        "b_full": nc.dram_tensor("b_full", [O, R], F32, kind="ExternalInput").ap(),
        "a_full": nc.dram_tensor("a_full", [R, D], F32, kind="ExternalInput").ap(),
        "mag_shard": nc.dram_tensor(
            "mag_shard", [O_SH], F32, kind="ExternalInput"
        ).ap(),
        "out_shard": nc.dram_tensor(
            "out_shard", [T_LOC, O], F32, kind="ExternalOutput"
        ).ap(),
    }
    with tile.TileContext(nc) as tc:
        _emit(nc, tc, aps)
    nc.compile()
    return nc


def run(inputs: dict, trace: bool = False):
    """Run the SPMD kernel on full inputs; returns (full_output, BassKernelResults)."""
    if "nc" not in _CACHE:
        _CACHE["nc"] = _build()
    nc = _CACHE["nc"]

    x = np.asarray(inputs["x"], dtype=np.float32).reshape(T, D)
    base = np.asarray(inputs["base_output"], dtype=np.float32).reshape(T, O)
    w = np.asarray(inputs["base_weight"], dtype=np.float32)
    a = np.ascontiguousarray(np.asarray(inputs["lora_A"], dtype=np.float32))
    b = np.ascontiguousarray(np.asarray(inputs["lora_B"], dtype=np.float32))
    mag = np.asarray(inputs["magnitude"], dtype=np.float32)

    in_maps = []
    for c in range(N_CORES):
        in_maps.append(
            {
                "x_shard": np.ascontiguousarray(x[c * T_LOC : (c + 1) * T_LOC]),
                "base_shard": np.ascontiguousarray(base[c * T_LOC : (c + 1) * T_LOC]),
                "w_shard": np.ascontiguousarray(w[c * O_SH : (c + 1) * O_SH]),
                "b_shard": np.ascontiguousarray(b[c * O_SH : (c + 1) * O_SH]),
                "b_full": b,
                "a_full": a,
                "mag_shard": np.ascontiguousarray(mag[c * O_SH : (c + 1) * O_SH]),
            }
        )

    res = run_bass_kernel_spmd(
        nc, in_maps, core_ids=list(range(N_CORES)), trace=trace
    )
    out = np.concatenate(
        [res.results[c]["out_shard"] for c in range(N_CORES)], axis=0
    )
    return out, res


def kernel(**inputs) -> np.ndarray:
    x = inputs["x"]
    out, _ = run(inputs)
    return out.reshape(x.shape[0], x.shape[1], O).astype(np.float32)


# revision 9
# speedup vs baseline: 1.4842x; 1.0524x over previous
"""DoRA linear kernel for 8 Trainium2 NeuronCores.

out = (base_output + 2.0 * x @ lora_A^T @ lora_B^T) * magnitude / (||base_weight + 2.0 * lora_B @ lora_A||_row + eps)

Sharding:
  - tokens (B*S = 8192) data-parallel: 1024 per core (x, base_output, out)
  - base_weight / lora_B / magnitude row-parallel: 512 out_features per core
    (per-row norm fully local; mag_scale allgathered, 16KB collective)
  - lora_A and lora_B replicated for the activation path

Precision: the low-rank delta path (x @ A^T @ B^T and B@A) runs in bf16 on
the PE (fp32 matmuls are dual-pass LOW_HIGH = 2x slower); the base_output
add, base_weight norm accumulation, and magnitude rescale stay fp32.

Engine / DMA-ring assignment (each engine's instruction stream is FIFO):
  - sync  (SP)  ring: magnitude, base tiles, output stores
  - scalar(ACT) ring: x tiles (fp32) -- ACT also casts x to bf16, copies
                      transpose/xa PSUMs to SBUF
  - gpsimd SWDGE:     lora_A/B, W tiles, collective in/out, mag broadcast;
                      GpSimd also runs half the epilogue multiplies
  - vector:           stage-0 norm adds + fused square-reduce, epilogue adds,
                      the other half of the multiplies
"""

import sys

sys.path.insert(0, "/opt/trn_rl_repo")

import numpy as np

import concourse.bass as bass  # noqa: F401
import concourse.mybir as mybir
import concourse.tile as tile
from concourse import bacc
from concourse.bass_utils import run_bass_kernel_spmd
from concourse.masks import make_identity

N_CORES = 8
T, D, O, R = 8192, 4096, 4096, 64
T_LOC = T // N_CORES  # 1024 tokens per core
O_SH = O // N_CORES  # 512 weight rows per core
SCALING = 2.0
EPS = 1e-8
F32 = mybir.dt.float32
BF16 = mybir.dt.bfloat16

N_TB = T_LOC // 128  # 8 token blocks per core
N_OC = O_SH // 128  # 4 o-chunks per core (stage 0)
N_DC512 = D // 512  # 8 d-chunks of 512
N_DC128 = D // 128  # 32 d-chunks of 128

_CACHE: dict = {}


def _emit(nc, tc, aps):
    x_d = aps["x_shard"]
    base_d = aps["base_shard"]
    w_d = aps["w_shard"]
    b_sh_d = aps["b_shard"]
    b_full_d = aps["b_full"]
    a_d = aps["a_full"]
    mag_d = aps["mag_shard"]
    out_d = aps["out_shard"]

    import contextlib

    ctx = contextlib.ExitStack()
    with ctx:
        const = ctx.enter_context(tc.tile_pool(name="const", bufs=1))
        wpool = ctx.enter_context(tc.tile_pool(name="wpool", bufs=2))
        x32pool = ctx.enter_context(tc.tile_pool(name="x32pool", bufs=2))
        x16pool = ctx.enter_context(tc.tile_pool(name="x16pool", bufs=2))
        bpool = ctx.enter_context(tc.tile_pool(name="bpool", bufs=5))
        xtpool = ctx.enter_context(tc.tile_pool(name="xtpool", bufs=2))
        xapool = ctx.enter_context(tc.tile_pool(name="xapool", bufs=8))
        opool = ctx.enter_context(tc.tile_pool(name="opool", bufs=4))
        scpool = ctx.enter_context(tc.tile_pool(name="scpool", bufs=2))
        p_u = ctx.enter_context(tc.tile_pool(name="p_u", bufs=2, space="PSUM"))
        p_t = ctx.enter_context(tc.tile_pool(name="p_t", bufs=2, space="PSUM"))
        p_xa = ctx.enter_context(tc.tile_pool(name="p_xa", bufs=1, space="PSUM"))
        p_o = ctx.enter_context(tc.tile_pool(name="p_o", bufs=3, space="PSUM"))
        dram = ctx.enter_context(tc.tile_pool(name="dram", bufs=1, space="DRAM"))

        # ---- constants / identity
        ident = const.tile([128, 128], BF16)
        make_identity(nc, ident[:])

        # ---- phase A: DMA triggers.  sync ring: magnitude + base halves.
        magsh_sb = const.tile([128, 4], F32)  # magnitude shard [p, oc]
        nc.sync.dma_start(magsh_sb[:], mag_d.rearrange("(oc p) -> p oc", p=128))

        x_r = x_d.rearrange("(tb p) d -> tb p d", p=128)
        base_r = base_d.rearrange("(tb p) d -> tb p d", p=128)
        out_r = out_d.rearrange("(tb p) d -> tb p d", p=128)
        w_r = w_d.rearrange("(oc p) d -> oc p d", p=128)

        base_tiles = {}
        for tb in range(N_TB):
            for h in range(2):
                bt = bpool.tile([128, 2048], F32, tag="base", name=f"base_{tb}_{h}")
                nc.sync.dma_start(bt[:], base_r[tb, :, 2048 * h : 2048 * (h + 1)])
                base_tiles[(tb, h)] = bt

        # gpsimd (SWDGE) ring: lora_A/B and W, all cast to bf16 on the fly
        a_sb = const.tile([R, D], BF16)  # lora_A natural [64, 4096]
        nc.gpsimd.dma_start(a_sb[:], a_d[:])
        bf_sb = const.tile([128, 32, R], BF16)  # lora_B rows chunked
        nc.gpsimd.dma_start(bf_sb[:], b_full_d.rearrange("(c p) r -> p c r", p=128))
        bs_sb = const.tile([128, 4, R], BF16)  # lora_B shard rows chunked
        nc.gpsimd.dma_start(bs_sb[:], b_sh_d.rearrange("(c p) r -> p c r", p=128))
        w_tiles = []
        for oc in range(N_OC):
            wt = wpool.tile([128, D], BF16, tag="w", name=f"w_{oc}")
            nc.gpsimd.dma_start(wt[:], w_r[oc])
            w_tiles.append(wt)

        # scalar ring: x tiles (fp32 halves)
        x32_tiles = {}

        def load_x(tb):
            for h in range(2):
                t = x32pool.tile([128, 2048], F32, tag="x32", name=f"x32_{tb}_{h}")
                nc.scalar.dma_start(t[:], x_r[tb, :, 2048 * h : 2048 * (h + 1)])
                x32_tiles[(tb, h)] = t

        load_x(0)
        load_x(1)

        # ---- phase B: preprocessing transposes (bf16)
        at_sb = const.tile([128, 64 * N_DC128], BF16)
        for g in range(4):  # 8 chunks per psum tile
            pt = p_t.tile([128, 512], BF16, tag="pt", name=f"pta_{g}")
            for j in range(8):
                dc = 8 * g + j
                nc.tensor.transpose(
                    pt[:, 64 * j : 64 * (j + 1)],
                    a_sb[:, 128 * dc : 128 * (dc + 1)],
                    ident[0:R, 0:R],
                )
            nc.scalar.copy(at_sb[:, 512 * g : 512 * (g + 1)], pt[:])

        b2ft_sb = const.tile([R, O], BF16)  # 2 * lora_B^T
        for g in range(8):
            pt = p_t.tile([128, 512], BF16, tag="pt", name=f"ptb_{g}")
            for j in range(4):
                c = 4 * g + j
                nc.tensor.transpose(
                    pt[0:R, 128 * j : 128 * (j + 1)], bf_sb[:, c, :], ident[:]
                )
            nc.scalar.mul(b2ft_sb[:, 512 * g : 512 * (g + 1)], pt[0:R, :], SCALING)

        b2st_sb = const.tile([R, O_SH], BF16)  # 2 * (lora_B shard)^T
        ptc = p_t.tile([128, 512], BF16, tag="pt")
        for j in range(4):
            nc.tensor.transpose(
                ptc[0:R, 128 * j : 128 * (j + 1)], bs_sb[:, j, :], ident[:]
            )
        nc.scalar.mul(b2st_sb[:], ptc[0:R, :], SCALING)

        # ---- helpers
        ss_sb = const.tile([128, N_OC, N_DC512], F32)
        magsc_sb = const.tile([128, 4], F32)
        magb_sb = const.tile([128, O], F32)

        def emit_stage1(tb):
            """xa^T[64, 128] = A @ x_tb^T via PE-transposed bf16 x chunks."""
            pxa = p_xa.tile([R, 128], F32, tag="pxa", name=f"pxa_{tb}")
            for h in range(2):
                x32 = x32_tiles.pop((tb, h))
                x16 = x16pool.tile([128, 2048], BF16, tag="x16", name=f"x16_{tb}_{h}")
                nc.scalar.copy(x16[:], x32[:])
                for g in range(4):
                    pt = p_t.tile([128, 512], BF16, tag="pt", name=f"ptx_{tb}_{h}_{g}")
                    for j in range(4):
                        nc.tensor.transpose(
                            pt[:, 128 * j : 128 * (j + 1)],
                            x16[:, 128 * (4 * g + j) : 128 * (4 * g + j + 1)],
                            ident[:],
                        )
                    xt = xtpool.tile([128, 512], BF16, tag="xt", name=f"xt_{tb}_{h}_{g}")
                    nc.scalar.copy(xt[:], pt[:])
                    for j in range(4):
                        dc = 16 * h + 4 * g + j
                        nc.tensor.matmul(
                            pxa[:],
                            at_sb[:, 64 * dc : 64 * (dc + 1)],
                            xt[:, 128 * j : 128 * (j + 1)],
                            start=(dc == 0),
                            stop=(dc == N_DC128 - 1),
                        )
            xa_sb = xapool.tile([R, 128], BF16, tag="xa", name=f"xa_{tb}")
            nc.scalar.copy(xa_sb[:], pxa[:])
            return xa_sb

        def emit_stage0_oc(oc):
            """sum-of-squares rows for one o-chunk of W + 2*B@A (all bf16)."""
            for dc in range(N_DC512):
                pu = p_u.tile([128, 512], F32, tag="pu", name=f"pu_{oc}_{dc}")
                nc.tensor.matmul(
                    pu[:],
                    b2st_sb[:, 128 * oc : 128 * (oc + 1)],
                    a_sb[:, 512 * dc : 512 * (dc + 1)],
                    start=True,
                    stop=False,
                )
                nc.tensor.matmul(
                    pu[:],
                    ident[:],
                    w_tiles[oc][:, 512 * dc : 512 * (dc + 1)],
                    start=False,
                    stop=True,
                )
                sq = scpool.tile([128, 512], BF16, tag="sq", name=f"sq_{oc}_{dc}")
                nc.scalar.activation(
                    sq[:],
                    pu[:],
                    mybir.ActivationFunctionType.Square,
                    accum_out=ss_sb[:, oc, dc : dc + 1],
                )

        def emit_stage0_tail():
            for oc in range(N_OC):
                ssum = scpool.tile([128, 1], F32, tag="ssum", name=f"ssum_{oc}")
                nc.vector.tensor_reduce(
                    ssum[:],
                    ss_sb[:, oc, :],
                    axis=mybir.AxisListType.X,
                    op=mybir.AluOpType.add,
                )
                nrm = scpool.tile([128, 1], F32, tag="nrm", name=f"nrm_{oc}")
                nc.scalar.sqrt(nrm[:], ssum[:])
                nc.vector.tensor_scalar_add(nrm[:], nrm[:], EPS)
                rinv = scpool.tile([128, 1], F32, tag="rinv", name=f"rinv_{oc}")
                nc.vector.reciprocal(rinv[:], nrm[:])
                nc.vector.tensor_tensor(
                    out=magsc_sb[:, oc : oc + 1],
                    in0=rinv[:],
                    in1=magsh_sb[:, oc : oc + 1],
                    op=mybir.AluOpType.mult,
                )

        def emit_collective():
            cc_in = dram.tile([O_SH], F32)
            cc_out = dram.tile([O], F32, addr_space="Shared")
            nc.gpsimd.dma_start(cc_in.rearrange("(oc p) -> p oc", p=128), magsc_sb[:])
            nc.gpsimd.collective_compute(
                "AllGather",
                mybir.AluOpType.bypass,
                replica_groups=[list(range(N_CORES))],
                ins=[cc_in[:]],
                outs=[cc_out[:]],
            )
            # replicate mag_scale row across all 128 partitions (DMA broadcast)
            nc.gpsimd.dma_start(magb_sb[:], cc_out[None, :].partition_broadcast(128))

        osb_tiles = {}

        def emit_stage2_adds(tb, xa_sb):
            """delta matmuls + base add into the output tile (no mag yet)."""
            osb = opool.tile([128, D], F32, tag="o", name=f"osb_{tb}")
            osb_tiles[tb] = osb
            for h in range(2):
                pos = [
                    p_o.tile([128, 512], F32, tag="po", name=f"po_{tb}_{h}_{j}")
                    for j in range(4)
                ]
                for j in range(4):
                    och = 4 * h + j
                    nc.tensor.matmul(
                        pos[j][:],
                        xa_sb[:],
                        b2ft_sb[:, 512 * och : 512 * (och + 1)],
                        start=True,
                        stop=True,
                    )
                for j in range(4):
                    och = 4 * h + j
                    nc.vector.tensor_tensor(
                        out=osb[:, 512 * och : 512 * (och + 1)],
                        in0=pos[j][:],
                        in1=base_tiles.pop((tb, h))[:, 512 * j : 512 * (j + 1)]
                        if j == 3
                        else base_tiles[(tb, h)][:, 512 * j : 512 * (j + 1)],
                        op=mybir.AluOpType.add,
                    )

        def emit_mults(tb):
            """magnitude rescale in-place: DVE takes h=0, GpSimd h=1."""
            osb = osb_tiles[tb]
            for h in range(2):
                eng = nc.vector if h == 0 else nc.gpsimd
                for j in range(4):
                    och = 4 * h + j
                    eng.tensor_tensor(
                        out=osb[:, 512 * och : 512 * (och + 1)],
                        in0=osb[:, 512 * och : 512 * (och + 1)],
                        in1=magb_sb[:, 512 * och : 512 * (och + 1)],
                        op=mybir.AluOpType.mult,
                    )

        # ---- phase C: stage 1 for tb0-3 interleaved with stage-0 chunks
        xa_tiles = {}
        for tb in range(N_OC):
            if tb + 2 < N_TB:
                load_x(tb + 2)
            xa_tiles[tb] = emit_stage1(tb)
            emit_stage0_oc(tb)

        # ---- phase D: stage-0 tails + collective
        emit_stage0_tail()
        emit_collective()

        # ---- phase E: stage-2 + adds for tb0-3
        for tb in range(N_OC):
            emit_stage2_adds(tb, xa_tiles[tb])

        # ---- phase F: remaining token blocks (stage1 + stage2 + adds)
        for tb in range(N_OC, N_TB):
            if tb + 2 < N_TB:
                load_x(tb + 2)
            xa_tiles[tb] = emit_stage1(tb)
            emit_stage2_adds(tb, xa_tiles[tb])

        # ---- phase G: mag rescale + output stores (store triggers at the
        # tail of the sync stream, after every base-load trigger)
        for tb in range(N_TB):
            emit_mults(tb)
        for tb in range(N_TB):
            nc.sync.dma_start(out_r[tb], osb_tiles[tb][:])


def _build():
    nc = bacc.Bacc(
        "TRN2", target_bir_lowering=False, debug=False, num_devices=N_CORES
    )
    aps = {
        "x_shard": nc.dram_tensor("x_shard", [T_LOC, D], F32, kind="ExternalInput").ap(),
        "base_shard": nc.dram_tensor(
            "base_shard", [T_LOC, O], F32, kind="ExternalInput"
        ).ap(),
        "w_shard": nc.dram_tensor("w_shard", [O_SH, D], F32, kind="ExternalInput").ap(),
        "b_shard": nc.dram_tensor("b_shard", [O_SH, R], F32, kind="ExternalInput").ap(),
        "b_full": nc.dram_tensor("b_full", [O, R], F32, kind="ExternalInput").ap(),
        "a_full": nc.dram_tensor("a_full", [R, D], F32, kind="ExternalInput").ap(),
        "mag_shard": nc.dram_tensor(
            "mag_shard", [O_SH], F32, kind="ExternalInput"
        ).ap(),
        "out_shard": nc.dram_tensor(
            "out_shard", [T_LOC, O], F32, kind="ExternalOutput"
        ).ap(),
    }
    with tile.TileContext(nc) as tc:
        _emit(nc, tc, aps)
    nc.compile()
    return nc


def run(inputs: dict, trace: bool = False):
    """Run the SPMD kernel on full inputs; returns (full_output, BassKernelResults)."""
    if "nc" not in _CACHE:
        _CACHE["nc"] = _build()
    nc = _CACHE["nc"]

    x = np.asarray(inputs["x"], dtype=np.float32).reshape(T, D)
    base = np.asarray(inputs["base_output"], dtype=np.float32).reshape(T, O)
    w = np.asarray(inputs["base_weight"], dtype=np.float32)
    a = np.ascontiguousarray(np.asarray(inputs["lora_A"], dtype=np.float32))
    b = np.ascontiguousarray(np.asarray(inputs["lora_B"], dtype=np.float32))
    mag = np.asarray(inputs["magnitude"], dtype=np.float32)

    in_maps = []
    for c in range(N_CORES):
        in_maps.append(
            {
                "x_shard": np.ascontiguousarray(x[c * T_LOC : (c + 1) * T_LOC]),
                "base_shard": np.ascontiguousarray(base[c * T_LOC : (c + 1) * T_LOC]),
                "w_shard": np.ascontiguousarray(w[c * O_SH : (c + 1) * O_SH]),
                "b_shard": np.ascontiguousarray(b[c * O_SH : (c + 1) * O_SH]),
                "b_full": b,
                "a_full": a,
                "mag_shard": np.ascontiguousarray(mag[c * O_SH : (c + 1) * O_SH]),
            }
        )

    res = run_bass_kernel_spmd(
        nc, in_maps, core_ids=list(range(N_CORES)), trace=trace
    )
    out = np.concatenate(
        [res.results[c]["out_shard"] for c in range(N_CORES)], axis=0
    )
    return out, res


def kernel(**inputs) -> np.ndarray:
    x = inputs["x"]
    out, _ = run(inputs)
    return out.reshape(x.shape[0], x.shape[1], O).astype(np.float32)
